# revision 6
# baseline (speedup 1.0000x reference)
"""Self-contained Trainium2 Bass kernel for nn_ChebNet_4320737100467.

ChebNet (K=2, two ChebConv layers + log_softmax) on a random graph with
N=100000 nodes, E=3200000 edges, sharded over 8 NeuronCores by destination
node. The separable symmetric normalization (w_e = -dis[row]*dis[col]) turns
edge aggregation into an unweighted gather-sum: rows are pre-scaled by dis
before projection and post-scaled by -dis after aggregation.

Per core: destination nodes are permuted into in-degree classes (K padded to
multiples of 2) so each node owns a fixed window of K gather slots laid out
[partition = node%128, free blocks = window]. Projected features are stored
in packed bf16 tables (4 nodes x 32 feats = one 256B row for layer A, 8 x 16
for layer B) so int16 dma_gather indices cover all permuted rows; the low
bits of the position select a 32/16-wide band after the gather.

v2 changes vs the original baseline: gather calls are batched to ~14K
indices (tile-aligned greedy packing) instead of 1024, cutting per-call
SWDGE fixed overhead ~15x; all DVE work concurrent with gathers avoids
2-port perf-mode ops (tensor_scalar / tensor_copy SBUF-SBUF) which hold the
SBUF port pair and fully block GpSimd descriptor generation - band masks are
built with one tensor_tensor is_equal against a constant band-index pattern,
per-partition scalings run on the idle Scalar engine (activation scale/bias),
and the band fold uses contiguous tensor_tensor adds (128->64->32[->16])
followed by a single small strided tensor_reduce over k, replacing the
2-byte-stride reduce that ran at 1/4 DVE rate.
"""

import numpy as np
import ml_dtypes
import jax
from jax.sharding import Mesh, PartitionSpec, NamedSharding
from jax.experimental.shard_map import shard_map

import concourse.bass as bass
import concourse.bacc as bacc
import concourse.tile as tile
import concourse.mybir as mybir
from concourse.bass2jax import _bass_exec_p, partition_id_tensor, install_neuronx_cc_hook

N = 100000
E = 3200000
NCORES = 8
NPD = 12500  # real nodes per device
# Max gather blocks (128 idx each) per dma_gather call. The SWDGE descriptor
# ring holds 256 descriptors per engine (16KB carveout / 64B); a call posts
# nb*8+1 per engine, so nb must stay below 31 to avoid deadlocking the ring.
CALLB = 30


def build_call_plan(schedule):
    """Pack gather work into calls of <= CALLB blocks, splitting tiles into
    pieces where needed.

    Returns (tiles, calls):
      tiles = [(kv, tile_i, blk0)]
      calls = [(c0, nb, [(tile_i, kv, k0, kw, off_in_call)...])]
    """
    tiles = []
    tile_i = 0
    blk0 = 0
    for kv, t in schedule:
        for _ in range(t):
            tiles.append((kv, tile_i, blk0))
            tile_i += 1
            blk0 += kv
    calls = []
    cur = []
    c0 = 0
    nb = 0
    for kv, ti, b0 in tiles:
        k0 = 0
        while k0 < kv:
            kw = min(kv - k0, CALLB - nb)
            if kw == 0:
                calls.append((c0, nb, cur))
                c0 += nb
                nb = 0
                cur = []
                continue
            cur.append((ti, kv, k0, kw, nb))
            nb += kw
            k0 += kw
    if cur:
        calls.append((c0, nb, cur))
    return tiles, calls


def preprocess(edge_index: np.ndarray):
    row = edge_index[0].astype(np.int64)
    col = edge_index[1].astype(np.int64)
    deg_full = np.bincount(row, minlength=N)
    dis_full = np.where(
        deg_full > 0, 1.0 / np.sqrt(np.maximum(deg_full, 1.0)), 0.0
    ).astype(np.float32)

    dev = row // NPD
    per = []
    for d in range(NCORES):
        m = dev == d
        per.append((row[m] - d * NPD, col[m]))

    Ks, perms = [], []
    for d in range(NCORES):
        r_loc, _ = per[d]
        degd = np.bincount(r_loc, minlength=NPD)
        K = np.maximum(2, ((degd + 1) // 2) * 2)
        perms.append(np.argsort(K, kind="stable"))
        Ks.append(K)

    kvals = sorted(set(int(k) for K in Ks for k in np.unique(K)))
    schedule = []
    for kv in kvals:
        cnt = max(int((K == kv).sum()) for K in Ks)
        t = (cnt + 127) // 128
        if t > 0:
            schedule.append((kv, t))
    ntiles = sum(t for _, t in schedule)
    nblk = sum(kv * t for kv, t in schedule)
    rows = ntiles * 128
    tiles, calls = build_call_plan(schedule)
    kmax = max(kv for kv, _ in schedule)

    node_of_pos = np.full((NCORES, rows), -1, np.int64)
    for d in range(NCORES):
        K = Ks[d]
        pos = 0
        for kv, t in schedule:
            ids = perms[d][K[perms[d]] == kv]
            node_of_pos[d, pos : pos + len(ids)] = ids
            pos += t * 128
    pos_of_node = np.full((NCORES, NPD), -1, np.int64)
    for d in range(NCORES):
        real = node_of_pos[d] >= 0
        pos_of_node[d, node_of_pos[d][real]] = np.nonzero(real)[0]

    table_rows_A = NCORES * rows // 4
    table_rows_B = NCORES * rows // 8
    assert table_rows_A <= 32768 and table_rows_B <= 32768, (
        table_rows_A,
        table_rows_B,
    )

    idxA = np.zeros((NCORES, 128, nblk * 8), np.int16)
    idxB = np.zeros((NCORES, 128, nblk * 8), np.int16)
    bandA = np.full((NCORES, 128, nblk), 255, np.int64)
    bandB = np.full((NCORES, 128, nblk), 255, np.int64)

    for d in range(NCORES):
        r_loc, c_glob = per[d]
        order = np.argsort(r_loc, kind="stable")
        r_s = r_loc[order]
        estart = np.searchsorted(r_s, np.arange(NPD + 1))
        gpos_col = (c_glob // NPD) * rows + pos_of_node[c_glob // NPD, c_glob % NPD]
        gpos_s = gpos_col[order]

        grid = np.full((128, nblk), -1, np.int64)
        for kv, tile_i, blk0 in tiles:
            nodes = node_of_pos[d, tile_i * 128 : (tile_i + 1) * 128]
            for p in range(128):
                nd = nodes[p]
                if nd < 0:
                    continue
                s0, s1 = estart[nd], estart[nd + 1]
                assert s1 - s0 <= kv
                grid[p, blk0 : blk0 + (s1 - s0)] = gpos_s[s0:s1]

        valid = grid >= 0
        ia = np.where(valid, grid >> 2, 0)
        ib = np.where(valid, grid >> 3, 0)
        bandA[d] = np.where(valid, grid & 3, 255)
        bandB[d] = np.where(valid, grid & 7, 255)

        for c0, nb, _tl in calls:
            sub_a = ia[:, c0 : c0 + nb]  # [128, nb]
            sub_b = ib[:, c0 : c0 + nb]
            flat_a = sub_a.T.reshape(-1)  # i = blk_local*128 + p
            flat_b = sub_b.T.reshape(-1)
            wrapped_a = np.tile(flat_a.reshape(-1, 16).T, (8, 1))  # [128, nb*8]
            wrapped_b = np.tile(flat_b.reshape(-1, 16).T, (8, 1))
            idxA[d, :, c0 * 8 : (c0 + nb) * 8] = wrapped_a
            idxB[d, :, c0 * 8 : (c0 + nb) * 8] = wrapped_b

    assert int(idxA.max()) < table_rows_A and int(idxB.max()) < table_rows_B

    return dict(
        dis_full=dis_full,
        schedule=schedule,
        tiles=tiles,
        calls=calls,
        kmax=kmax,
        ntiles_total=ntiles,
        nblk_total=nblk,
        rows_per_dev=rows,
        node_of_pos=node_of_pos,
        idxA=idxA,
        idxB=idxB,
        bandA=bandA,
        bandB=bandB,
    )


def build_core_inputs(x, dis_full, meta, d):
    nop = meta["node_of_pos"][d]
    rows = len(nop)
    xp = np.zeros((rows, 128), np.float32)
    disp = np.zeros(rows, np.float32)
    real = nop >= 0
    gids = nop[real] + d * NPD
    xp[real] = np.asarray(x)[gids]
    disp[real] = dis_full[gids]
    nt = meta["ntiles_total"]
    dis2d = np.ascontiguousarray(disp.reshape(nt, 128).T)
    return xp, dis2d


# ---------------- kernel builder ----------------
F32 = mybir.dt.float32
BF16 = mybir.dt.bfloat16
I16 = mybir.dt.int16
AX = mybir.AxisListType
ALU = mybir.AluOpType
ACT = mybir.ActivationFunctionType


def build(meta):
    schedule = meta["schedule"]
    ntiles = meta["ntiles_total"]
    rows = meta["rows_per_dev"]
    tiles = meta["tiles"]
    calls = meta["calls"]
    kmax = meta["kmax"]
    nblk = meta["nblk_total"]
    ta_rows = NCORES * rows // 4
    tb_rows = NCORES * rows // 8

    nc = bacc.Bacc(
        "TRN2",
        target_bir_lowering=False,
        debug=False,
        num_devices=NCORES,
    )
    x_in = nc.declare_dram_parameter("x_perm", [rows, 128], F32, isOutput=False)
    dis_in = nc.declare_dram_parameter("dis2d", [128, ntiles], F32, isOutput=False)
    w01a_in = nc.declare_dram_parameter("W01a", [128, 64], F32, isOutput=False)
    w01b_in = nc.declare_dram_parameter("W01b", [32, 32], F32, isOutput=False)
    ba_in = nc.declare_dram_parameter("ba2", [128, 32], F32, isOutput=False)
    bb_in = nc.declare_dram_parameter("bb2", [128, 16], F32, isOutput=False)
    eye_in = nc.declare_dram_parameter("eye", [128, 128], F32, isOutput=False)
    jp4_in = nc.declare_dram_parameter("jpat4", [128, kmax * 4], BF16, isOutput=False)
    jp8_in = nc.declare_dram_parameter("jpat8", [128, kmax * 8], BF16, isOutput=False)
    idxa_in = nc.declare_dram_parameter("idxA2", [128, nblk * 8], I16, isOutput=False)
    idxb_in = nc.declare_dram_parameter("idxB2", [128, nblk * 8], I16, isOutput=False)
    banda_in = nc.declare_dram_parameter("bandA", [128, nblk], BF16, isOutput=False)
    bandb_in = nc.declare_dram_parameter("bandB", [128, nblk], BF16, isOutput=False)
    out_ext = nc.declare_dram_parameter("out_perm", [rows, 16], F32, isOutput=True)

    pa_slice = nc.dram_tensor("pa_slice", [rows // 4, 128], BF16)
    pb_slice = nc.dram_tensor("pb_slice", [rows // 8, 128], BF16)
    pa_table = nc.dram_tensor("pa_table", [ta_rows, 128], BF16, addr_space="Shared")
    pb_table = nc.dram_tensor("pb_table", [tb_rows, 128], BF16, addr_space="Shared")

    groups = [list(range(NCORES))]

    with tile.TileContext(nc) as tc:
        with (
            tc.tile_pool(name="const", bufs=1) as cpool,
            tc.tile_pool(name="work", bufs=3) as wpool,
            tc.tile_pool(name="gath", bufs=3) as gpool,
            tc.tile_pool(name="fold", bufs=3) as fpool,
            tc.tile_pool(name="tred", bufs=6) as tpool,
            tc.tile_pool(name="psum", bufs=2, space="PSUM") as ppool,
            tc.tile_pool(name="psum1", bufs=2, space="PSUM") as ppool1,
        ):
            # ---- constants / residents
            w01a = cpool.tile([128, 64], F32)
            nc.sync.dma_start(out=w01a[:], in_=w01a_in[:])
            w01b = cpool.tile([32, 32], F32)
            nc.sync.dma_start(out=w01b[:], in_=w01b_in[:])
            ba_sb = cpool.tile([128, 32], F32)
            nc.sync.dma_start(out=ba_sb[:], in_=ba_in[:])
            bb_sb = cpool.tile([128, 16], F32)
            nc.sync.dma_start(out=bb_sb[:], in_=bb_in[:])
            eye = cpool.tile([128, 128], F32)
            nc.sync.dma_start(out=eye[:], in_=eye_in[:])
            dis_sb = cpool.tile([128, ntiles], F32)
            nc.sync.dma_start(out=dis_sb[:], in_=dis_in[:])
            # -dis via Scalar engine (avoid DVE tensor_scalar 2-port mode)
            negdis = cpool.tile([128, ntiles], F32)
            nc.scalar.activation(negdis[:], dis_sb[:], ACT.Copy, scale=-1.0)
            jp4 = cpool.tile([128, kmax * 4], BF16)
            nc.sync.dma_start(out=jp4[:], in_=jp4_in[:])
            jp8 = cpool.tile([128, kmax * 8], BF16)
            nc.sync.dma_start(out=jp8[:], in_=jp8_in[:])
            qb_sb = cpool.tile([128, ntiles * 32], F32)
            hwb_sb = cpool.tile([128, ntiles * 16], F32)
            banda_sb = cpool.tile([128, nblk], BF16)
            nc.sync.dma_start(out=banda_sb[:], in_=banda_in[:])
            bandb_sb = cpool.tile([128, nblk], BF16)
            nc.sync.dma_start(out=bandb_sb[:], in_=bandb_in[:])

            # ---- S1: projections over own tiles
            for kv, t, blk0 in tiles:
                xt = wpool.tile([128, 128], F32, tag="xtile")
                nc.sync.dma_start(out=xt[:], in_=x_in[t * 128 : (t + 1) * 128, :])
                xT_ps = ppool.tile([128, 128], F32, tag="xT")
                nc.tensor.transpose(out=xT_ps[:], in_=xt[:], identity=eye[:])
                xT = wpool.tile([128, 128], F32, tag="xT_sb")
                nc.vector.tensor_copy(out=xT[:], in_=xT_ps[:])
                p01 = ppool1.tile([128, 64], F32, tag="p01")
                nc.tensor.matmul(p01[:], lhsT=xT[:], rhs=w01a[:], start=True, stop=True)
                # qb = x@W0a + ba  (tensor_tensor: non-contending)
                nc.vector.tensor_tensor(
                    out=qb_sb[:, t * 32 : (t + 1) * 32],
                    in0=p01[:, 0:32],
                    in1=ba_sb[:],
                    op=ALU.add,
                )
                # pa = dis * (x@W1a), cast to bf16 on Scalar engine
                pa_bf = wpool.tile([128, 32], BF16, tag="pa_bf")
                nc.scalar.activation(
                    pa_bf[:], p01[:, 32:64], ACT.Copy, scale=dis_sb[:, t : t + 1]
                )
                nc.sync.dma_start(
                    out=pa_slice[t * 32 : (t + 1) * 32, :].rearrange(
                        "r (g f) -> (r g) f", g=4
                    ),
                    in_=pa_bf[:],
                )

            # ---- S2: allgather pa
            nc.gpsimd.collective_compute(
                "AllGather",
                ALU.bypass,
                replica_groups=groups,
                ins=[pa_slice[:, :]],
                outs=[pa_table[:, :]],
            )

            # ---- S3 + S5 helper: batched gather + per-piece band fold + reduce
            def edge_phase(layer, table, idx_in, band_sb, jpat, nband, fw):
                live = {}  # tile_i -> accumulated partial tred
                for c0, nb, plist in calls:
                    chunk = wpool.tile([128, CALLB * 8], I16, tag="idx")
                    nc.sync.dma_start(
                        out=chunk[:, 0 : nb * 8],
                        in_=idx_in[:, c0 * 8 : (c0 + nb) * 8],
                    )
                    gt = gpool.tile([128, CALLB, 128], BF16, tag="g")
                    nc.gpsimd.dma_gather(
                        gt[:, 0:nb, :],
                        table[:, :],
                        chunk[:, 0 : nb * 8],
                        nb * 128,
                        nb * 128,
                        128,
                        single_packet=False,
                    )
                    for t_i, kv, k0, kw, off in plist:
                        blk0 = c0 + off  # global block index of this piece
                        # band mask via tensor_tensor is_equal (1-port mode)
                        mask = wpool.tile([128, CALLB * 8], BF16, tag="m")
                        nc.vector.tensor_tensor(
                            out=mask[:, 0 : kw * nband].rearrange(
                                "p (k j) -> p k j", j=nband
                            ),
                            in0=band_sb[:, blk0 : blk0 + kw].to_broadcast(
                                [128, kw, nband]
                            ),
                            in1=jpat[:, 0 : kw * nband].rearrange(
                                "p (k j) -> p k j", j=nband
                            ),
                            op=ALU.is_equal,
                        )
                        # masked multiply in place; contiguous iteration
                        gview = gt[:, off : off + kw, :].rearrange(
                            "p k (j f) -> p (k j) f", f=fw
                        )
                        nc.vector.tensor_tensor(
                            out=gview,
                            in0=gview,
                            in1=mask[:, 0 : kw * nband].to_broadcast(
                                [128, kw * nband, fw]
                            ),
                            op=ALU.mult,
                        )
                        # fold bands pairwise: 128 -> 64 -> 32 (-> 16)
                        th = fpool.tile([128, CALLB * 64], BF16, tag="th")
                        nc.vector.tensor_tensor(
                            out=th[:, 0 : kw * 64],
                            in0=gt[:, off : off + kw, 0:64],
                            in1=gt[:, off : off + kw, 64:128],
                            op=ALU.add,
                        )
                        thv = th[:, 0 : kw * 64].rearrange("p (k f) -> p k f", f=64)
                        tq = fpool.tile([128, CALLB * 32], BF16, tag="tq")
                        nc.vector.tensor_tensor(
                            out=tq[:, 0 : kw * 32],
                            in0=thv[:, :, 0:32],
                            in1=thv[:, :, 32:64],
                            op=ALU.add,
                        )
                        if fw == 16:
                            tqv = tq[:, 0 : kw * 32].rearrange(
                                "p (k f) -> p k f", f=32
                            )
                            te = fpool.tile([128, CALLB * 16], BF16, tag="te")
                            nc.vector.tensor_tensor(
                                out=te[:, 0 : kw * 16],
                                in0=tqv[:, :, 0:16],
                                in1=tqv[:, :, 16:32],
                                op=ALU.add,
                            )
                            red_in = te[:, 0 : kw * 16].rearrange(
                                "p (k f) -> p f k", f=16
                            )
                        else:
                            red_in = tq[:, 0 : kw * 32].rearrange(
                                "p (k f) -> p f k", f=32
                            )
                        if t_i not in live:
                            tred = tpool.tile([128, fw], F32, tag="tr")
                            nc.vector.tensor_reduce(tred[:], red_in, AX.X, ALU.add)
                            live[t_i] = tred
                        else:
                            tred = live[t_i]
                            part = tpool.tile([128, fw], F32, tag="trp")
                            nc.vector.tensor_reduce(part[:], red_in, AX.X, ALU.add)
                            nc.vector.tensor_tensor(
                                out=tred[:], in0=tred[:], in1=part[:], op=ALU.add
                            )
                        if k0 + kw == kv:
                            del live[t_i]
                            yield kv, t_i, tred

            # ---- S3: layer A edge phase + h + layer B projections
            for kv, t, t1 in edge_phase("a", pa_table, idxa_in, banda_sb, jp4, 4, 32):
                # z = qb - dis * t1 ; scale on Scalar engine, add on DVE
                t1s = wpool.tile([128, 32], F32, tag="t1s")
                nc.scalar.activation(
                    t1s[:], t1[:], ACT.Copy, scale=negdis[:, t : t + 1]
                )
                z = wpool.tile([128, 32], F32, tag="z1")
                nc.vector.tensor_tensor(
                    out=z[:], in0=t1s[:], in1=qb_sb[:, t * 32 : (t + 1) * 32],
                    op=ALU.add,
                )
                h = wpool.tile([128, 32], F32, tag="h")
                nc.scalar.activation(h[:], z[:], ACT.Relu)
                hT_ps = ppool.tile([32, 128], F32, tag="hT")
                nc.tensor.transpose(out=hT_ps[:], in_=h[:], identity=eye[:])
                hT = wpool.tile([32, 128], F32, tag="hT_sb")
                nc.vector.tensor_copy(out=hT[:], in_=hT_ps[:])
                pb01 = ppool1.tile([128, 32], F32, tag="pb01")
                nc.tensor.matmul(
                    pb01[:], lhsT=hT[:], rhs=w01b[:], start=True, stop=True
                )
                nc.vector.tensor_tensor(
                    out=hwb_sb[:, t * 16 : (t + 1) * 16],
                    in0=pb01[:, 0:16],
                    in1=bb_sb[:],
                    op=ALU.add,
                )
                pb_bf = wpool.tile([128, 16], BF16, tag="pb_bf")
                nc.scalar.activation(
                    pb_bf[:], pb01[:, 16:32], ACT.Copy, scale=dis_sb[:, t : t + 1]
                )
                nc.sync.dma_start(
                    out=pb_slice[t * 16 : (t + 1) * 16, :].rearrange(
                        "r (g f) -> (r g) f", g=8
                    ),
                    in_=pb_bf[:],
                )

            # ---- S4: allgather pb
            nc.gpsimd.collective_compute(
                "AllGather",
                ALU.bypass,
                replica_groups=groups,
                ins=[pb_slice[:, :]],
                outs=[pb_table[:, :]],
            )

            # ---- S5: layer B edge phase + log_softmax tail
            for kv, t, t2 in edge_phase("b", pb_table, idxb_in, bandb_sb, jp8, 8, 16):
                t2s = wpool.tile([128, 16], F32, tag="t2s")
                nc.scalar.activation(
                    t2s[:], t2[:], ACT.Copy, scale=negdis[:, t : t + 1]
                )
                z = wpool.tile([128, 16], F32, tag="z2")
                nc.vector.tensor_tensor(
                    out=z[:], in0=t2s[:], in1=hwb_sb[:, t * 16 : (t + 1) * 16],
                    op=ALU.add,
                )
                negmx = wpool.tile([128, 1], F32, tag="mx")
                nc.vector.tensor_reduce(negmx[:], z[:], AX.X, ALU.max, negate=True)
                zc = wpool.tile([128, 16], F32, tag="zc")
                nc.scalar.activation(zc[:], z[:], ACT.Identity, bias=negmx[:, 0:1])
                ex = wpool.tile([128, 16], F32, tag="ex")
                sm = wpool.tile([128, 1], F32, tag="sm")
                nc.scalar.activation(ex[:], zc[:], ACT.Exp, accum_out=sm[:])
                ls = wpool.tile([128, 1], F32, tag="ls")
                nc.scalar.activation(ls[:], sm[:], ACT.Ln)
                ot = wpool.tile([128, 16], F32, tag="ot")
                nc.vector.tensor_tensor(
                    out=ot[:], in0=zc[:], in1=ls[:, 0:1].to_broadcast([128, 16]),
                    op=ALU.subtract,
                )
                nc.sync.dma_start(
                    out=out_ext[t * 128 : (t + 1) * 128, :], in_=ot[:]
                )

    nc.finalize()
    return nc


# ---------------- runner ----------------
class SpmdRunner:
    def __init__(self, nc: bass.Bass, n_cores: int):
        install_neuronx_cc_hook()
        self.nc = nc
        self.n_cores = n_cores
        partition_name = nc.partition_id_tensor.name if nc.partition_id_tensor else None
        in_names, out_names, out_avals = [], [], []
        for alloc in nc.m.functions[0].allocations:
            if not isinstance(alloc, mybir.MemoryLocationSet):
                continue
            name = alloc.memorylocations[0].name
            if alloc.kind == "ExternalInput":
                if name != partition_name:
                    in_names.append(name)
            elif alloc.kind == "ExternalOutput":
                out_names.append(name)
                out_avals.append(
                    jax.core.ShapedArray(
                        tuple(alloc.tensor_shape), mybir.dt.np(alloc.dtype)
                    )
                )
        self.in_names = list(in_names)
        self.out_names = out_names
        self.out_avals = out_avals
        n_params = len(in_names)
        all_names = in_names + out_names
        if partition_name is not None:
            all_names.append(partition_name)
        self.partition_name = partition_name

        def _body(*args):
            operands = list(args)
            if partition_name is not None:
                operands.append(partition_id_tensor())
            return tuple(
                _bass_exec_p.bind(
                    *operands,
                    out_avals=tuple(out_avals),
                    in_names=tuple(all_names),
                    out_names=tuple(out_names),
                    lowering_input_output_aliases=(),
                    sim_require_finite=True,
                    sim_require_nnan=True,
                    nc=nc,
                )
            )

        devices = jax.devices()[:n_cores]
        assert len(devices) == n_cores
        self.mesh = Mesh(np.asarray(devices), ("core",))
        n_io = n_params + len(out_names)
        self.fn = jax.jit(
            shard_map(
                _body,
                mesh=self.mesh,
                in_specs=(PartitionSpec("core"),) * n_io,
                out_specs=(PartitionSpec("core"),) * len(out_names),
                check_rep=False,
            ),
            keep_unused=True,
        )
        self.sharding = NamedSharding(self.mesh, PartitionSpec("core"))
        self._dev_in = None

    def put_inputs(self, in_maps: list[dict[str, np.ndarray]]):
        assert len(in_maps) == self.n_cores
        concat = [
            np.concatenate([np.asarray(m[name]) for m in in_maps], axis=0)
            for name in self.in_names
        ]
        zeros = [
            np.zeros((self.n_cores * a.shape[0], *a.shape[1:]), a.dtype)
            for a in self.out_avals
        ]
        self._dev_in = [jax.device_put(a, self.sharding) for a in concat + zeros]
        return self

    def run(self):
        outs = self.fn(*self._dev_in)
        jax.block_until_ready(outs)
        return outs

    def results(self, outs) -> list[dict[str, np.ndarray]]:
        res = []
        for c in range(self.n_cores):
            d = {}
            for i, name in enumerate(self.out_names):
                full = np.asarray(outs[i])
                per = full.reshape(self.n_cores, *self.out_avals[i].shape)
                d[name] = per[c]
            res.append(d)
        return res


# ---------------- driver / entry point ----------------
def make_in_maps(inputs, meta):
    x = np.asarray(inputs["x"], np.float32)
    W0a = np.asarray(inputs["W0a"], np.float32)
    W1a = np.asarray(inputs["W1a"], np.float32)
    W0b = np.asarray(inputs["W0b"], np.float32)
    W1b = np.asarray(inputs["W1b"], np.float32)
    ba = np.asarray(inputs["ba"], np.float32)
    bb = np.asarray(inputs["bb"], np.float32)
    w01a = np.concatenate([W0a, W1a], axis=1)  # [128, 64]
    w01b = np.concatenate([W0b, W1b], axis=1)  # [32, 32]
    eye = np.eye(128, dtype=np.float32)
    bf = ml_dtypes.bfloat16
    kmax = meta["kmax"]
    jp4 = np.tile(np.arange(4, dtype=np.float32), kmax)
    jp4 = np.tile(jp4.reshape(1, -1), (128, 1)).astype(bf)
    jp8 = np.tile(np.arange(8, dtype=np.float32), kmax)
    jp8 = np.tile(jp8.reshape(1, -1), (128, 1)).astype(bf)
    in_maps = []
    for d in range(NCORES):
        xp, dis2d = build_core_inputs(x, meta["dis_full"], meta, d)
        in_maps.append(
            dict(
                x_perm=xp,
                dis2d=dis2d,
                W01a=w01a,
                W01b=w01b,
                ba2=np.tile(ba.reshape(1, 32), (128, 1)),
                bb2=np.tile(bb.reshape(1, 16), (128, 1)),
                eye=eye,
                jpat4=jp4,
                jpat8=jp8,
                idxA2=meta["idxA"][d],
                idxB2=meta["idxB"][d],
                bandA=meta["bandA"][d].astype(np.float32).astype(bf),
                bandB=meta["bandB"][d].astype(np.float32).astype(bf),
            )
        )
    return in_maps


def unpermute(outs, meta):
    """outs: list of per-core out_perm [rows, 16] -> [N, 16]."""
    out_full = np.zeros((N, 16), np.float32)
    for d in range(NCORES):
        nop = meta["node_of_pos"][d]
        real = nop >= 0
        out_full[nop[real] + d * NPD] = np.asarray(outs[d])[np.nonzero(real)[0]]
    return out_full


_CACHE = {}


def kernel(**inputs) -> np.ndarray:
    edge_index = np.asarray(inputs["edge_index"])
    key = edge_index.tobytes()[:4096]
    if key not in _CACHE:
        meta = preprocess(edge_index)
        nc = build(meta)
        runner = SpmdRunner(nc, NCORES)
        _CACHE[key] = (meta, runner)
    meta, runner = _CACHE[key]
    in_maps = make_in_maps(inputs, meta)
    runner.put_inputs(in_maps)
    outs = runner.run()
    res = runner.results(outs)
    return unpermute([res[d]["out_perm"] for d in range(NCORES)], meta)


# revision 8
# speedup vs baseline: 1.0161x; 1.0161x over previous
"""Self-contained Trainium2 Bass kernel for nn_ChebNet_4320737100467.

ChebNet (K=2, two ChebConv layers + log_softmax) on a random graph with
N=100000 nodes, E=3200000 edges, sharded over 8 NeuronCores by destination
node. The separable symmetric normalization (w_e = -dis[row]*dis[col]) turns
edge aggregation into an unweighted gather-sum: rows are pre-scaled by dis
before projection and post-scaled by -dis after aggregation.

Per core: destination nodes are permuted into in-degree classes (K padded to
multiples of 2) so each node owns a fixed window of K gather slots laid out
[partition = node%128, free blocks = window]. Projected features are stored
in packed bf16 tables (4 nodes x 32 feats = one 256B row for layer A, 8 x 16
for layer B) so int16 dma_gather indices cover all permuted rows; the low
bits of the position select a 32/16-wide band after the gather.

v2 changes vs the original baseline: gather calls are batched to ~14K
indices (tile-aligned greedy packing) instead of 1024, cutting per-call
SWDGE fixed overhead ~15x; all DVE work concurrent with gathers avoids
2-port perf-mode ops (tensor_scalar / tensor_copy SBUF-SBUF) which hold the
SBUF port pair and fully block GpSimd descriptor generation - band masks are
built with one tensor_tensor is_equal against a constant band-index pattern,
per-partition scalings run on the idle Scalar engine (activation scale/bias),
and the band fold uses contiguous tensor_tensor adds (128->64->32[->16])
followed by a single small strided tensor_reduce over k, replacing the
2-byte-stride reduce that ran at 1/4 DVE rate.
"""

import numpy as np
import ml_dtypes
import jax
from jax.sharding import Mesh, PartitionSpec, NamedSharding
from jax.experimental.shard_map import shard_map

import concourse.bass as bass
import concourse.bacc as bacc
import concourse.tile as tile
import concourse.mybir as mybir
from concourse.bass2jax import _bass_exec_p, partition_id_tensor, install_neuronx_cc_hook

N = 100000
E = 3200000
NCORES = 8
NPD = 12500  # real nodes per device
# Max gather blocks (128 idx each) per dma_gather call. The SWDGE descriptor
# ring holds 256 descriptors per engine (16KB carveout / 64B); a call posts
# nb*8+1 per engine, so nb must stay below 31 to avoid deadlocking the ring.
CALLB = 30


def build_call_plan(schedule):
    """Pack gather work into calls of <= CALLB blocks, splitting tiles into
    pieces where needed.

    Returns (tiles, calls):
      tiles = [(kv, tile_i, blk0)]
      calls = [(c0, nb, [(tile_i, kv, k0, kw, off_in_call)...])]
    """
    tiles = []
    tile_i = 0
    blk0 = 0
    for kv, t in schedule:
        for _ in range(t):
            tiles.append((kv, tile_i, blk0))
            tile_i += 1
            blk0 += kv
    calls = []
    cur = []
    c0 = 0
    nb = 0
    for kv, ti, b0 in tiles:
        k0 = 0
        while k0 < kv:
            kw = min(kv - k0, CALLB - nb)
            if kw == 0:
                calls.append((c0, nb, cur))
                c0 += nb
                nb = 0
                cur = []
                continue
            cur.append((ti, kv, k0, kw, nb))
            nb += kw
            k0 += kw
    if cur:
        calls.append((c0, nb, cur))
    return tiles, calls


def preprocess(edge_index: np.ndarray):
    row = edge_index[0].astype(np.int64)
    col = edge_index[1].astype(np.int64)
    deg_full = np.bincount(row, minlength=N)
    dis_full = np.where(
        deg_full > 0, 1.0 / np.sqrt(np.maximum(deg_full, 1.0)), 0.0
    ).astype(np.float32)

    dev = row // NPD
    per = []
    for d in range(NCORES):
        m = dev == d
        per.append((row[m] - d * NPD, col[m]))

    # Degree classes chosen by DP over the joint degree histogram to minimize
    # total padded gather slots: cost(a,b] = b * 128 * ceil(max_core_count/128).
    degs = []
    maxdeg = 1
    for d in range(NCORES):
        r_loc, _ = per[d]
        degd = np.maximum(np.bincount(r_loc, minlength=NPD), 1)
        degs.append(degd)
        maxdeg = max(maxdeg, int(degd.max()))
    cum = np.zeros((NCORES, maxdeg + 1), np.int64)
    for d in range(NCORES):
        h = np.bincount(degs[d], minlength=maxdeg + 1)[: maxdeg + 1]
        cum[d] = np.cumsum(h)
    INF = 1 << 60
    dp = [INF] * (maxdeg + 1)
    dp[0] = 0
    parent = [-1] * (maxdeg + 1)
    for b in range(1, maxdeg + 1):
        for a in range(0, b):
            m = int((cum[:, b] - cum[:, a]).max())
            c = dp[a] + b * 128 * ((m + 127) // 128)
            if c < dp[b]:
                dp[b] = c
                parent[b] = a
    bounds = []
    b = maxdeg
    while b > 0:
        bounds.append(b)
        b = parent[b]
    bounds = np.array(sorted(bounds), np.int64)

    Ks, perms = [], []
    for d in range(NCORES):
        K = bounds[np.searchsorted(bounds, degs[d])]
        perms.append(np.argsort(K, kind="stable"))
        Ks.append(K)

    kvals = [int(v) for v in bounds]
    schedule = []
    for kv in kvals:
        cnt = max(int((K == kv).sum()) for K in Ks)
        t = (cnt + 127) // 128
        if t > 0:
            schedule.append((kv, t))
    ntiles = sum(t for _, t in schedule)
    nblk = sum(kv * t for kv, t in schedule)
    rows = ntiles * 128
    tiles, calls = build_call_plan(schedule)
    kmax = max(kv for kv, _ in schedule)

    node_of_pos = np.full((NCORES, rows), -1, np.int64)
    for d in range(NCORES):
        K = Ks[d]
        pos = 0
        for kv, t in schedule:
            ids = perms[d][K[perms[d]] == kv]
            node_of_pos[d, pos : pos + len(ids)] = ids
            pos += t * 128
    pos_of_node = np.full((NCORES, NPD), -1, np.int64)
    for d in range(NCORES):
        real = node_of_pos[d] >= 0
        pos_of_node[d, node_of_pos[d][real]] = np.nonzero(real)[0]

    table_rows_A = NCORES * rows // 4
    table_rows_B = NCORES * rows // 8
    assert table_rows_A <= 32768 and table_rows_B <= 32768, (
        table_rows_A,
        table_rows_B,
    )

    idxA = np.zeros((NCORES, 128, nblk * 8), np.int16)
    idxB = np.zeros((NCORES, 128, nblk * 8), np.int16)
    bandA = np.full((NCORES, 128, nblk), 255, np.int64)
    bandB = np.full((NCORES, 128, nblk), 255, np.int64)

    for d in range(NCORES):
        r_loc, c_glob = per[d]
        order = np.argsort(r_loc, kind="stable")
        r_s = r_loc[order]
        estart = np.searchsorted(r_s, np.arange(NPD + 1))
        gpos_col = (c_glob // NPD) * rows + pos_of_node[c_glob // NPD, c_glob % NPD]
        gpos_s = gpos_col[order]

        grid = np.full((128, nblk), -1, np.int64)
        for kv, tile_i, blk0 in tiles:
            nodes = node_of_pos[d, tile_i * 128 : (tile_i + 1) * 128]
            for p in range(128):
                nd = nodes[p]
                if nd < 0:
                    continue
                s0, s1 = estart[nd], estart[nd + 1]
                assert s1 - s0 <= kv
                grid[p, blk0 : blk0 + (s1 - s0)] = gpos_s[s0:s1]

        valid = grid >= 0
        ia = np.where(valid, grid >> 2, 0)
        ib = np.where(valid, grid >> 3, 0)
        bandA[d] = np.where(valid, grid & 3, 255)
        bandB[d] = np.where(valid, grid & 7, 255)

        for c0, nb, _tl in calls:
            sub_a = ia[:, c0 : c0 + nb]  # [128, nb]
            sub_b = ib[:, c0 : c0 + nb]
            flat_a = sub_a.T.reshape(-1)  # i = blk_local*128 + p
            flat_b = sub_b.T.reshape(-1)
            wrapped_a = np.tile(flat_a.reshape(-1, 16).T, (8, 1))  # [128, nb*8]
            wrapped_b = np.tile(flat_b.reshape(-1, 16).T, (8, 1))
            idxA[d, :, c0 * 8 : (c0 + nb) * 8] = wrapped_a
            idxB[d, :, c0 * 8 : (c0 + nb) * 8] = wrapped_b

    assert int(idxA.max()) < table_rows_A and int(idxB.max()) < table_rows_B

    return dict(
        dis_full=dis_full,
        schedule=schedule,
        tiles=tiles,
        calls=calls,
        kmax=kmax,
        ntiles_total=ntiles,
        nblk_total=nblk,
        rows_per_dev=rows,
        node_of_pos=node_of_pos,
        idxA=idxA,
        idxB=idxB,
        bandA=bandA,
        bandB=bandB,
    )


def build_core_inputs(x, dis_full, meta, d):
    nop = meta["node_of_pos"][d]
    rows = len(nop)
    xp = np.zeros((rows, 128), np.float32)
    disp = np.zeros(rows, np.float32)
    real = nop >= 0
    gids = nop[real] + d * NPD
    xp[real] = np.asarray(x)[gids]
    disp[real] = dis_full[gids]
    nt = meta["ntiles_total"]
    dis2d = np.ascontiguousarray(disp.reshape(nt, 128).T)
    return xp, dis2d


# ---------------- kernel builder ----------------
F32 = mybir.dt.float32
BF16 = mybir.dt.bfloat16
I16 = mybir.dt.int16
AX = mybir.AxisListType
ALU = mybir.AluOpType
ACT = mybir.ActivationFunctionType


def build(meta):
    schedule = meta["schedule"]
    ntiles = meta["ntiles_total"]
    rows = meta["rows_per_dev"]
    tiles = meta["tiles"]
    calls = meta["calls"]
    kmax = meta["kmax"]
    nblk = meta["nblk_total"]
    ta_rows = NCORES * rows // 4
    tb_rows = NCORES * rows // 8

    nc = bacc.Bacc(
        "TRN2",
        target_bir_lowering=False,
        debug=False,
        num_devices=NCORES,
    )
    x_in = nc.declare_dram_parameter("x_perm", [rows, 128], F32, isOutput=False)
    dis_in = nc.declare_dram_parameter("dis2d", [128, ntiles], F32, isOutput=False)
    w01a_in = nc.declare_dram_parameter("W01a", [128, 64], F32, isOutput=False)
    w01b_in = nc.declare_dram_parameter("W01b", [32, 32], F32, isOutput=False)
    ba_in = nc.declare_dram_parameter("ba2", [128, 32], F32, isOutput=False)
    bb_in = nc.declare_dram_parameter("bb2", [128, 16], F32, isOutput=False)
    eye_in = nc.declare_dram_parameter("eye", [128, 128], F32, isOutput=False)
    jp4_in = nc.declare_dram_parameter("jpat4", [128, kmax * 4], BF16, isOutput=False)
    jp8_in = nc.declare_dram_parameter("jpat8", [128, kmax * 8], BF16, isOutput=False)
    idxa_in = nc.declare_dram_parameter("idxA2", [128, nblk * 8], I16, isOutput=False)
    idxb_in = nc.declare_dram_parameter("idxB2", [128, nblk * 8], I16, isOutput=False)
    banda_in = nc.declare_dram_parameter("bandA", [128, nblk], BF16, isOutput=False)
    bandb_in = nc.declare_dram_parameter("bandB", [128, nblk], BF16, isOutput=False)
    out_ext = nc.declare_dram_parameter("out_perm", [rows, 16], F32, isOutput=True)

    pa_slice = nc.dram_tensor("pa_slice", [rows // 4, 128], BF16)
    pb_slice = nc.dram_tensor("pb_slice", [rows // 8, 128], BF16)
    pa_table = nc.dram_tensor("pa_table", [ta_rows, 128], BF16, addr_space="Shared")
    pb_table = nc.dram_tensor("pb_table", [tb_rows, 128], BF16, addr_space="Shared")

    groups = [list(range(NCORES))]

    with tile.TileContext(nc) as tc:
        with (
            tc.tile_pool(name="const", bufs=1) as cpool,
            tc.tile_pool(name="work", bufs=4) as wpool,
            tc.tile_pool(name="gath", bufs=5) as gpool,
            tc.tile_pool(name="fold", bufs=4) as fpool,
            tc.tile_pool(name="tred", bufs=8) as tpool,
            tc.tile_pool(name="psum", bufs=2, space="PSUM") as ppool,
            tc.tile_pool(name="psum1", bufs=2, space="PSUM") as ppool1,
        ):
            # ---- constants / residents
            w01a = cpool.tile([128, 64], F32)
            nc.sync.dma_start(out=w01a[:], in_=w01a_in[:])
            w01b = cpool.tile([32, 32], F32)
            nc.sync.dma_start(out=w01b[:], in_=w01b_in[:])
            ba_sb = cpool.tile([128, 32], F32)
            nc.sync.dma_start(out=ba_sb[:], in_=ba_in[:])
            bb_sb = cpool.tile([128, 16], F32)
            nc.sync.dma_start(out=bb_sb[:], in_=bb_in[:])
            eye = cpool.tile([128, 128], F32)
            nc.sync.dma_start(out=eye[:], in_=eye_in[:])
            dis_sb = cpool.tile([128, ntiles], F32)
            nc.sync.dma_start(out=dis_sb[:], in_=dis_in[:])
            # -dis via Scalar engine (avoid DVE tensor_scalar 2-port mode)
            negdis = cpool.tile([128, ntiles], F32)
            nc.scalar.activation(negdis[:], dis_sb[:], ACT.Copy, scale=-1.0)
            jp4 = cpool.tile([128, kmax * 4], BF16)
            nc.sync.dma_start(out=jp4[:], in_=jp4_in[:])
            jp8 = cpool.tile([128, kmax * 8], BF16)
            nc.sync.dma_start(out=jp8[:], in_=jp8_in[:])
            qb_sb = cpool.tile([128, ntiles * 32], F32)
            hwb_sb = cpool.tile([128, ntiles * 16], F32)
            banda_sb = cpool.tile([128, nblk], BF16)
            nc.sync.dma_start(out=banda_sb[:], in_=banda_in[:])
            bandb_sb = cpool.tile([128, nblk], BF16)
            nc.sync.dma_start(out=bandb_sb[:], in_=bandb_in[:])

            # ---- S1: projections over own tiles (x loaded 4 tiles per DMA)
            xq = {}
            for kv, t, blk0 in tiles:
                if t % 4 == 0:
                    nload = min(4, ntiles - t)
                    xt4 = wpool.tile([128, 4, 128], F32, tag="xtile")
                    nc.sync.dma_start(
                        out=xt4[:, 0:nload, :],
                        in_=x_in[t * 128 : (t + nload) * 128, :].rearrange(
                            "(b r) c -> r b c", r=128
                        ),
                    )
                    xq[t // 4] = xt4
                xt4 = xq[t // 4]
                xT_ps = ppool.tile([128, 128], F32, tag="xT")
                nc.tensor.transpose(
                    out=xT_ps[:], in_=xt4[:, t % 4, :], identity=eye[:]
                )
                xT = wpool.tile([128, 128], F32, tag="xT_sb")
                nc.vector.tensor_copy(out=xT[:], in_=xT_ps[:])
                p01 = ppool1.tile([128, 64], F32, tag="p01")
                nc.tensor.matmul(p01[:], lhsT=xT[:], rhs=w01a[:], start=True, stop=True)
                # qb = x@W0a + ba  (tensor_tensor: non-contending)
                nc.vector.tensor_tensor(
                    out=qb_sb[:, t * 32 : (t + 1) * 32],
                    in0=p01[:, 0:32],
                    in1=ba_sb[:],
                    op=ALU.add,
                )
                # pa = dis * (x@W1a), cast to bf16 on Scalar engine
                pa_bf = wpool.tile([128, 32], BF16, tag="pa_bf")
                nc.scalar.activation(
                    pa_bf[:], p01[:, 32:64], ACT.Copy, scale=dis_sb[:, t : t + 1]
                )
                nc.scalar.dma_start(
                    out=pa_slice[t * 32 : (t + 1) * 32, :].rearrange(
                        "r (g f) -> (r g) f", g=4
                    ),
                    in_=pa_bf[:],
                )

            # ---- S2: allgather pa
            nc.gpsimd.collective_compute(
                "AllGather",
                ALU.bypass,
                replica_groups=groups,
                ins=[pa_slice[:, :]],
                outs=[pa_table[:, :]],
            )

            # ---- S3 + S5 helper: batched gather + per-piece band fold + reduce
            def edge_phase(layer, table, idx_in, band_sb, jpat, nband, fw):
                live = {}  # tile_i -> accumulated partial tred
                for c0, nb, plist in calls:
                    chunk = wpool.tile([128, CALLB * 8], I16, tag="idx")
                    nc.sync.dma_start(
                        out=chunk[:, 0 : nb * 8],
                        in_=idx_in[:, c0 * 8 : (c0 + nb) * 8],
                    )
                    gt = gpool.tile([128, CALLB, 128], BF16, tag="g")
                    nc.gpsimd.dma_gather(
                        gt[:, 0:nb, :],
                        table[:, :],
                        chunk[:, 0 : nb * 8],
                        nb * 128,
                        nb * 128,
                        128,
                        single_packet=False,
                    )
                    for t_i, kv, k0, kw, off in plist:
                        blk0 = c0 + off  # global block index of this piece
                        # band mask via tensor_tensor is_equal (1-port mode)
                        mask = wpool.tile([128, CALLB * 8], BF16, tag="m")
                        nc.vector.tensor_tensor(
                            out=mask[:, 0 : kw * nband].rearrange(
                                "p (k j) -> p k j", j=nband
                            ),
                            in0=band_sb[:, blk0 : blk0 + kw].to_broadcast(
                                [128, kw, nband]
                            ),
                            in1=jpat[:, 0 : kw * nband].rearrange(
                                "p (k j) -> p k j", j=nband
                            ),
                            op=ALU.is_equal,
                        )
                        # masked multiply in place; contiguous iteration
                        gview = gt[:, off : off + kw, :].rearrange(
                            "p k (j f) -> p (k j) f", f=fw
                        )
                        nc.vector.tensor_tensor(
                            out=gview,
                            in0=gview,
                            in1=mask[:, 0 : kw * nband].to_broadcast(
                                [128, kw * nband, fw]
                            ),
                            op=ALU.mult,
                        )
                        # fold bands pairwise: 128 -> 64 -> 32 (-> 16)
                        th = fpool.tile([128, CALLB * 64], BF16, tag="th")
                        nc.vector.tensor_tensor(
                            out=th[:, 0 : kw * 64],
                            in0=gt[:, off : off + kw, 0:64],
                            in1=gt[:, off : off + kw, 64:128],
                            op=ALU.add,
                        )
                        thv = th[:, 0 : kw * 64].rearrange("p (k f) -> p k f", f=64)
                        tq = fpool.tile([128, CALLB * 32], BF16, tag="tq")
                        nc.vector.tensor_tensor(
                            out=tq[:, 0 : kw * 32],
                            in0=thv[:, :, 0:32],
                            in1=thv[:, :, 32:64],
                            op=ALU.add,
                        )
                        if fw == 16:
                            tqv = tq[:, 0 : kw * 32].rearrange(
                                "p (k f) -> p k f", f=32
                            )
                            te = fpool.tile([128, CALLB * 16], BF16, tag="te")
                            nc.vector.tensor_tensor(
                                out=te[:, 0 : kw * 16],
                                in0=tqv[:, :, 0:16],
                                in1=tqv[:, :, 16:32],
                                op=ALU.add,
                            )
                            red_in = te[:, 0 : kw * 16].rearrange(
                                "p (k f) -> p f k", f=16
                            )
                        else:
                            red_in = tq[:, 0 : kw * 32].rearrange(
                                "p (k f) -> p f k", f=32
                            )
                        if t_i not in live:
                            tred = tpool.tile([128, fw], F32, tag="tr")
                            nc.vector.tensor_reduce(tred[:], red_in, AX.X, ALU.add)
                            live[t_i] = tred
                        else:
                            tred = live[t_i]
                            part = tpool.tile([128, fw], F32, tag="trp")
                            nc.vector.tensor_reduce(part[:], red_in, AX.X, ALU.add)
                            nc.vector.tensor_tensor(
                                out=tred[:], in0=tred[:], in1=part[:], op=ALU.add
                            )
                        if k0 + kw == kv:
                            del live[t_i]
                            yield kv, t_i, tred

            # ---- S3: layer A edge phase + h + layer B projections
            for kv, t, t1 in edge_phase("a", pa_table, idxa_in, banda_sb, jp4, 4, 32):
                # z = qb - dis * t1 ; scale on Scalar engine, add on DVE
                t1s = wpool.tile([128, 32], F32, tag="t1s")
                nc.scalar.activation(
                    t1s[:], t1[:], ACT.Copy, scale=negdis[:, t : t + 1]
                )
                z = wpool.tile([128, 32], F32, tag="z1")
                nc.vector.tensor_tensor(
                    out=z[:], in0=t1s[:], in1=qb_sb[:, t * 32 : (t + 1) * 32],
                    op=ALU.add,
                )
                h = wpool.tile([128, 32], F32, tag="h")
                nc.scalar.activation(h[:], z[:], ACT.Relu)
                hT_ps = ppool.tile([32, 128], F32, tag="hT")
                nc.tensor.transpose(out=hT_ps[:], in_=h[:], identity=eye[:])
                hT = wpool.tile([32, 128], F32, tag="hT_sb")
                nc.vector.tensor_copy(out=hT[:], in_=hT_ps[:])
                pb01 = ppool1.tile([128, 32], F32, tag="pb01")
                nc.tensor.matmul(
                    pb01[:], lhsT=hT[:], rhs=w01b[:], start=True, stop=True
                )
                nc.vector.tensor_tensor(
                    out=hwb_sb[:, t * 16 : (t + 1) * 16],
                    in0=pb01[:, 0:16],
                    in1=bb_sb[:],
                    op=ALU.add,
                )
                pb_bf = wpool.tile([128, 16], BF16, tag="pb_bf")
                nc.scalar.activation(
                    pb_bf[:], pb01[:, 16:32], ACT.Copy, scale=dis_sb[:, t : t + 1]
                )
                nc.scalar.dma_start(
                    out=pb_slice[t * 16 : (t + 1) * 16, :].rearrange(
                        "r (g f) -> (r g) f", g=8
                    ),
                    in_=pb_bf[:],
                )

            # ---- S4: allgather pb
            nc.gpsimd.collective_compute(
                "AllGather",
                ALU.bypass,
                replica_groups=groups,
                ins=[pb_slice[:, :]],
                outs=[pb_table[:, :]],
            )

            # ---- S5: layer B edge phase + log_softmax tail
            for kv, t, t2 in edge_phase("b", pb_table, idxb_in, bandb_sb, jp8, 8, 16):
                t2s = wpool.tile([128, 16], F32, tag="t2s")
                nc.scalar.activation(
                    t2s[:], t2[:], ACT.Copy, scale=negdis[:, t : t + 1]
                )
                z = wpool.tile([128, 16], F32, tag="z2")
                nc.vector.tensor_tensor(
                    out=z[:], in0=t2s[:], in1=hwb_sb[:, t * 16 : (t + 1) * 16],
                    op=ALU.add,
                )
                negmx = wpool.tile([128, 1], F32, tag="mx")
                nc.vector.tensor_reduce(negmx[:], z[:], AX.X, ALU.max, negate=True)
                zc = wpool.tile([128, 16], F32, tag="zc")
                nc.scalar.activation(zc[:], z[:], ACT.Identity, bias=negmx[:, 0:1])
                ex = wpool.tile([128, 16], F32, tag="ex")
                sm = wpool.tile([128, 1], F32, tag="sm")
                nc.scalar.activation(ex[:], zc[:], ACT.Exp, accum_out=sm[:])
                ls = wpool.tile([128, 1], F32, tag="ls")
                nc.scalar.activation(ls[:], sm[:], ACT.Ln)
                ot = wpool.tile([128, 16], F32, tag="ot")
                nc.vector.tensor_tensor(
                    out=ot[:], in0=zc[:], in1=ls[:, 0:1].to_broadcast([128, 16]),
                    op=ALU.subtract,
                )
                nc.scalar.dma_start(
                    out=out_ext[t * 128 : (t + 1) * 128, :], in_=ot[:]
                )

    nc.finalize()
    return nc


# ---------------- runner ----------------
class SpmdRunner:
    def __init__(self, nc: bass.Bass, n_cores: int):
        install_neuronx_cc_hook()
        self.nc = nc
        self.n_cores = n_cores
        partition_name = nc.partition_id_tensor.name if nc.partition_id_tensor else None
        in_names, out_names, out_avals = [], [], []
        for alloc in nc.m.functions[0].allocations:
            if not isinstance(alloc, mybir.MemoryLocationSet):
                continue
            name = alloc.memorylocations[0].name
            if alloc.kind == "ExternalInput":
                if name != partition_name:
                    in_names.append(name)
            elif alloc.kind == "ExternalOutput":
                out_names.append(name)
                out_avals.append(
                    jax.core.ShapedArray(
                        tuple(alloc.tensor_shape), mybir.dt.np(alloc.dtype)
                    )
                )
        self.in_names = list(in_names)
        self.out_names = out_names
        self.out_avals = out_avals
        n_params = len(in_names)
        all_names = in_names + out_names
        if partition_name is not None:
            all_names.append(partition_name)
        self.partition_name = partition_name

        def _body(*args):
            operands = list(args)
            if partition_name is not None:
                operands.append(partition_id_tensor())
            return tuple(
                _bass_exec_p.bind(
                    *operands,
                    out_avals=tuple(out_avals),
                    in_names=tuple(all_names),
                    out_names=tuple(out_names),
                    lowering_input_output_aliases=(),
                    sim_require_finite=True,
                    sim_require_nnan=True,
                    nc=nc,
                )
            )

        devices = jax.devices()[:n_cores]
        assert len(devices) == n_cores
        self.mesh = Mesh(np.asarray(devices), ("core",))
        n_io = n_params + len(out_names)
        self.fn = jax.jit(
            shard_map(
                _body,
                mesh=self.mesh,
                in_specs=(PartitionSpec("core"),) * n_io,
                out_specs=(PartitionSpec("core"),) * len(out_names),
                check_rep=False,
            ),
            keep_unused=True,
        )
        self.sharding = NamedSharding(self.mesh, PartitionSpec("core"))
        self._dev_in = None

    def put_inputs(self, in_maps: list[dict[str, np.ndarray]]):
        assert len(in_maps) == self.n_cores
        concat = [
            np.concatenate([np.asarray(m[name]) for m in in_maps], axis=0)
            for name in self.in_names
        ]
        zeros = [
            np.zeros((self.n_cores * a.shape[0], *a.shape[1:]), a.dtype)
            for a in self.out_avals
        ]
        self._dev_in = [jax.device_put(a, self.sharding) for a in concat + zeros]
        return self

    def run(self):
        outs = self.fn(*self._dev_in)
        jax.block_until_ready(outs)
        return outs

    def results(self, outs) -> list[dict[str, np.ndarray]]:
        res = []
        for c in range(self.n_cores):
            d = {}
            for i, name in enumerate(self.out_names):
                full = np.asarray(outs[i])
                per = full.reshape(self.n_cores, *self.out_avals[i].shape)
                d[name] = per[c]
            res.append(d)
        return res


# ---------------- driver / entry point ----------------
def make_in_maps(inputs, meta):
    x = np.asarray(inputs["x"], np.float32)
    W0a = np.asarray(inputs["W0a"], np.float32)
    W1a = np.asarray(inputs["W1a"], np.float32)
    W0b = np.asarray(inputs["W0b"], np.float32)
    W1b = np.asarray(inputs["W1b"], np.float32)
    ba = np.asarray(inputs["ba"], np.float32)
    bb = np.asarray(inputs["bb"], np.float32)
    w01a = np.concatenate([W0a, W1a], axis=1)  # [128, 64]
    w01b = np.concatenate([W0b, W1b], axis=1)  # [32, 32]
    eye = np.eye(128, dtype=np.float32)
    bf = ml_dtypes.bfloat16
    kmax = meta["kmax"]
    jp4 = np.tile(np.arange(4, dtype=np.float32), kmax)
    jp4 = np.tile(jp4.reshape(1, -1), (128, 1)).astype(bf)
    jp8 = np.tile(np.arange(8, dtype=np.float32), kmax)
    jp8 = np.tile(jp8.reshape(1, -1), (128, 1)).astype(bf)
    in_maps = []
    for d in range(NCORES):
        xp, dis2d = build_core_inputs(x, meta["dis_full"], meta, d)
        in_maps.append(
            dict(
                x_perm=xp,
                dis2d=dis2d,
                W01a=w01a,
                W01b=w01b,
                ba2=np.tile(ba.reshape(1, 32), (128, 1)),
                bb2=np.tile(bb.reshape(1, 16), (128, 1)),
                eye=eye,
                jpat4=jp4,
                jpat8=jp8,
                idxA2=meta["idxA"][d],
                idxB2=meta["idxB"][d],
                bandA=meta["bandA"][d].astype(np.float32).astype(bf),
                bandB=meta["bandB"][d].astype(np.float32).astype(bf),
            )
        )
    return in_maps


def unpermute(outs, meta):
    """outs: list of per-core out_perm [rows, 16] -> [N, 16]."""
    out_full = np.zeros((N, 16), np.float32)
    for d in range(NCORES):
        nop = meta["node_of_pos"][d]
        real = nop >= 0
        out_full[nop[real] + d * NPD] = np.asarray(outs[d])[np.nonzero(real)[0]]
    return out_full


_CACHE = {}


def kernel(**inputs) -> np.ndarray:
    edge_index = np.asarray(inputs["edge_index"])
    key = edge_index.tobytes()[:4096]
    if key not in _CACHE:
        meta = preprocess(edge_index)
        nc = build(meta)
        runner = SpmdRunner(nc, NCORES)
        _CACHE[key] = (meta, runner)
    meta, runner = _CACHE[key]
    in_maps = make_in_maps(inputs, meta)
    runner.put_inputs(in_maps)
    outs = runner.run()
    res = runner.results(outs)
    return unpermute([res[d]["out_perm"] for d in range(NCORES)], meta)
